# revision 8
# baseline (speedup 1.0000x reference)
"""GCN layer (projection + gather/segment-sum + epilogue) on 8 TRN2 cores.

Math: out = relu((segsum(norm[src]*h[src] -> dst) * norm) @ wh + bh + m @ wm + bm)
using (h@wh)*n == (n*h)@wh to hoist both norm scalings into a per-edge weight
normboth[e] = norm[src[e]] * norm[dst[e]] that is folded into the one-hot
scatter matrix. Per core (dst-sharded, 12500 nodes):
  - dma_gather pulls h16[src] rows into edge tiles (128 edges x 128 feats);
    int16 gather indices are local to one of four 32768-row source chunks
  - one-hot scatter matrix built on DVE with one fused tensor_scalar
    (iota == dstmod) * normboth
  - PE: msgs.T @ onehot accumulates agg.T per 128-node dst block in PSUM
  - PE: wh.T @ agg.T + wm.T @ m.T -> out.T, ACT fuses bias+relu
Output is produced feature-major [128, nodes]; host transposes back.
"""

import math

import numpy as np

import concourse.bacc as bacc
import concourse.tile as tile
from concourse import bass, mybir
from concourse import bass_utils

N_NODES = 100000
N_EDGES = 1600000
F = 128
P = 128
N_CORES = 8
SHARD = N_NODES // N_CORES          # 12500 nodes per core
NBLK = math.ceil(SHARD / P)         # 98 blocks of 128 dst nodes
SHARD_PAD = NBLK * P                # 12544
SB_BLOCKS = 7                       # dst blocks per gather super-block
N_SB = math.ceil(NBLK / SB_BLOCKS)  # 14
CHUNK = 32768                       # int16-addressable source rows
N_CHUNKS = math.ceil(N_NODES / CHUNK)
GDT = mybir.dt.float16
GNP = np.float16


class Layout:
    """Tile-column layout shared by host packing and the device program.
    Cells (dst block, src chunk) are ordered super-block-major, then chunk,
    then block, so each (super-block, chunk) gather call is contiguous."""

    def __init__(self, caps):
        self.caps = caps                      # [NBLK][N_CHUNKS] tile counts
        self.cell_col = np.zeros((NBLK, N_CHUNKS), np.int64)
        self.sb_col = np.zeros(N_SB + 1, np.int64)
        col = 0
        for s in range(N_SB):
            self.sb_col[s] = col
            b_lo, b_hi = s * SB_BLOCKS, min((s + 1) * SB_BLOCKS, NBLK)
            for c in range(N_CHUNKS):
                for b in range(b_lo, b_hi):
                    self.cell_col[b, c] = col
                    col += caps[b, c]
        self.sb_col[N_SB] = col
        self.tot = int(col)
        # rank of each cell id (b * N_CHUNKS + c) in the layout order
        self.rank = np.zeros(NBLK * N_CHUNKS, np.int64)
        self.rank_col = np.zeros(NBLK * N_CHUNKS, np.int64)
        r = 0
        for s in range(N_SB):
            b_lo, b_hi = s * SB_BLOCKS, min((s + 1) * SB_BLOCKS, NBLK)
            for c in range(N_CHUNKS):
                for b in range(b_lo, b_hi):
                    self.rank[b * N_CHUNKS + c] = r
                    self.rank_col[r] = self.cell_col[b, c]
                    r += 1
        self.rank_caps = np.zeros(NBLK * N_CHUNKS, np.int64)
        self.rank_caps[self.rank] = caps.reshape(-1)


def _pack_edges(src, dst, norm_flat):
    core_of = dst // SHARD
    per_core = []
    counts_all = np.zeros((N_CORES, NBLK * N_CHUNKS), np.int64)
    for i in range(N_CORES):
        sel = np.nonzero(core_of == i)[0]
        es = src[sel].astype(np.int64)
        ed = (dst[sel] - i * SHARD).astype(np.int64)
        cell = (ed >> 7) * N_CHUNKS + (es // CHUNK)
        counts_all[i] = np.bincount(cell, minlength=NBLK * N_CHUNKS)
        per_core.append((es, ed, cell))

    caps = ((counts_all.max(axis=0) + P - 1) // P).reshape(NBLK, N_CHUNKS)
    assert caps.sum(axis=1).min() >= 1
    lay = Layout(caps)

    packed = []
    for i in range(N_CORES):
        es, ed, cell = per_core[i]
        rank = lay.rank[cell]
        order = np.argsort(rank, kind="stable")
        es, ed, rank = es[order], ed[order], rank[order]
        counts = np.bincount(rank, minlength=NBLK * N_CHUNKS)
        starts = np.concatenate([[0], np.cumsum(counts)])
        k = np.arange(len(es)) - starts[rank]
        col = lay.rank_col[rank] + (k >> 7)
        row = k & 127
        # int16 gather index local to the source chunk; pads point at row 0
        idx_slots = np.zeros(lay.tot * P, np.int16)
        idx_slots[col * P + row] = (es - (es // CHUNK) * CHUNK).astype(np.int16)
        wrapped = idx_slots.reshape(lay.tot * P // 16, 16).T
        idx16 = np.tile(wrapped, (8, 1))
        dmod = np.full((lay.tot, P), -1.0, np.float32)
        dmod[col, row] = (ed & 127).astype(np.float32)
        nb = np.zeros((lay.tot, P), np.float32)
        nb[col, row] = norm_flat[es] * norm_flat[ed + i * SHARD]
        packed.append((
            np.ascontiguousarray(idx16),
            np.ascontiguousarray(dmod.T),
            np.ascontiguousarray(nb.T),
        ))
    return packed, lay


def _build_program(lay):
    """One SPMD program; all 8 cores run it on their own data."""
    nc = bacc.Bacc(
        "TRN2", target_bir_lowering=False, debug=False, num_devices=N_CORES
    )
    f32 = mybir.dt.float32
    tot = lay.tot
    h_d = nc.dram_tensor("h16", [N_NODES, F], GDT, kind="ExternalInput").ap()
    idx_d = nc.dram_tensor("eidx", [P, tot * 8], mybir.dt.int16, kind="ExternalInput").ap()
    dmod_d = nc.dram_tensor("edmod", [P, tot], f32, kind="ExternalInput").ap()
    nb_d = nc.dram_tensor("enboth", [P, tot], f32, kind="ExternalInput").ap()
    mt_d = nc.dram_tensor("mT", [F, SHARD_PAD], f32, kind="ExternalInput").ap()
    wh_d = nc.dram_tensor("wh16", [F, F], GDT, kind="ExternalInput").ap()
    wm_d = nc.dram_tensor("wm16", [F, F], GDT, kind="ExternalInput").ap()
    iota_d = nc.dram_tensor("iota", [P, P], GDT, kind="ExternalInput").ap()
    bias_d = nc.dram_tensor("bias", [F, 1], f32, kind="ExternalInput").ap()
    out_d = nc.dram_tensor("outT", [F, SHARD_PAD], f32, kind="ExternalOutput").ap()

    t_max = int(max(lay.sb_col[s + 1] - lay.sb_col[s] for s in range(N_SB)))

    with tile.TileContext(nc) as tc:
        with (
            tc.tile_pool(name="const", bufs=1) as cpool,
            tc.tile_pool(name="msgs", bufs=1) as mpool,
            tc.tile_pool(name="oh", bufs=8) as ohpool,
            tc.tile_pool(name="agg", bufs=3) as aggpool,
            tc.tile_pool(name="mw", bufs=4) as mwpool,
            tc.tile_pool(name="outp", bufs=3) as opool,
            tc.tile_pool(name="pacc", bufs=2, space="PSUM") as paccp,
            tc.tile_pool(name="pout", bufs=2, space="PSUM") as poutp,
        ):
            idx_s = cpool.tile([P, tot * 8], mybir.dt.int16, tag="idx")
            dmod_s = cpool.tile([P, tot], f32, tag="dmod")
            nb_s = cpool.tile([P, tot], f32, tag="nb")
            wh_s = cpool.tile([F, F], GDT, tag="wh")
            wm_s = cpool.tile([F, F], GDT, tag="wm")
            iota_s = cpool.tile([P, P], GDT, tag="iota")
            bias_s = cpool.tile([F, 1], f32, tag="bias")
            nc.sync.dma_start(out=idx_s[:], in_=idx_d[:])
            nc.sync.dma_start(out=dmod_s[:], in_=dmod_d[:])
            nc.sync.dma_start(out=nb_s[:], in_=nb_d[:])
            nc.sync.dma_start(out=wh_s[:], in_=wh_d[:])
            nc.sync.dma_start(out=wm_s[:], in_=wm_d[:])
            nc.sync.dma_start(out=iota_s[:], in_=iota_d[:])
            nc.sync.dma_start(out=bias_s[:], in_=bias_d[:])

            msgs = [
                mpool.tile([P, t_max * F], GDT, tag="m0", name="msgs0"),
                mpool.tile([P, t_max * F], GDT, tag="m1", name="msgs1"),
            ]

            for s in range(N_SB):
                b_lo = s * SB_BLOCKS
                b_hi = min((s + 1) * SB_BLOCKS, NBLK)
                sc0 = int(lay.sb_col[s])
                mt = msgs[s % 2]
                for c in range(N_CHUNKS):
                    cc0 = int(lay.cell_col[b_lo, c])
                    ncols_all = int(sum(lay.caps[b, c] for b in range(b_lo, b_hi)))
                    rows = min(CHUNK, N_NODES - c * CHUNK)
                    # SWDGE ring limits one gather to <16384 descriptors
                    for p0 in range(0, ncols_all, 64):
                        ncols = min(64, ncols_all - p0)
                        g0 = cc0 + p0
                        ni = ncols * P
                        lo = (g0 - sc0) * F
                        nc.gpsimd.dma_gather(
                            out_ap=mt[:, lo : lo + ncols * F].rearrange(
                                "p (t d) -> p t d", d=F
                            ),
                            in_ap=h_d[c * CHUNK : c * CHUNK + rows, :],
                            idxs_ap=idx_s[:, g0 * 8 : g0 * 8 + ni // 16],
                            num_idxs=ni,
                            num_idxs_reg=ni,
                            elem_size=F,
                            single_packet=False,
                        )

                for b in range(b_lo, b_hi):
                    cols = []
                    for c in range(N_CHUNKS):
                        c0 = int(lay.cell_col[b, c])
                        cols.extend(range(c0, c0 + int(lay.caps[b, c])))
                    acc = paccp.tile([F, P], mybir.dt.float32, tag="acc")
                    for j, c_abs in enumerate(cols):
                        oh = ohpool.tile([P, P], GDT, tag="oh")
                        nc.vector.tensor_scalar(
                            out=oh[:],
                            in0=iota_s[:],
                            scalar1=dmod_s[:, c_abs : c_abs + 1],
                            scalar2=nb_s[:, c_abs : c_abs + 1],
                            op0=mybir.AluOpType.is_equal,
                            op1=mybir.AluOpType.mult,
                        )
                        lo = (c_abs - sc0) * F
                        nc.tensor.matmul(
                            acc[:],
                            lhsT=mt[:, lo : lo + F],
                            rhs=oh[:],
                            start=(j == 0),
                            stop=(j == len(cols) - 1),
                        )
                    agg16 = aggpool.tile([F, P], GDT, tag="agg")
                    nc.vector.tensor_copy(out=agg16[:], in_=acc[:])

                    mtf = mwpool.tile([F, P], f32, tag="mtf")
                    nc.sync.dma_start(
                        out=mtf[:], in_=mt_d[:, b * P : (b + 1) * P]
                    )
                    mt16 = mwpool.tile([F, P], GDT, tag="mt16")
                    nc.vector.tensor_copy(out=mt16[:], in_=mtf[:])

                    po = poutp.tile([F, P], mybir.dt.float32, tag="po")
                    nc.tensor.matmul(
                        po[:], lhsT=wh_s[:], rhs=agg16[:], start=True, stop=False
                    )
                    nc.tensor.matmul(
                        po[:], lhsT=wm_s[:], rhs=mt16[:], start=False, stop=True
                    )
                    ot = opool.tile([F, P], f32, tag="ot")
                    nc.scalar.activation(
                        out=ot[:],
                        in_=po[:],
                        func=mybir.ActivationFunctionType.Relu,
                        bias=bias_s[:],
                    )
                    nc.sync.dma_start(
                        out=out_d[:, b * P : (b + 1) * P], in_=ot[:]
                    )
    nc.compile()
    return nc


def kernel(h, m, norm, src, dst, wh, wm, bh, bm):
    h16 = np.ascontiguousarray(np.asarray(h, np.float32).astype(GNP))
    m = np.asarray(m, np.float32)
    norm_flat = np.asarray(norm, np.float32).reshape(-1)
    src = np.asarray(src, np.int32)
    dst = np.asarray(dst, np.int32)
    bias = (np.asarray(bh, np.float32) + np.asarray(bm, np.float32)).reshape(F, 1)

    packed, lay = _pack_edges(src, dst, norm_flat)

    wh16 = np.asarray(wh, np.float32).astype(GNP)
    wm16 = np.asarray(wm, np.float32).astype(GNP)
    iota = np.broadcast_to(np.arange(P, dtype=GNP), (P, P)).copy()

    in_maps = []
    for i in range(N_CORES):
        idx16, dmod_t, nb_t = packed[i]
        m_shard = np.zeros((F, SHARD_PAD), np.float32)
        m_shard[:, :SHARD] = m[i * SHARD : (i + 1) * SHARD].T
        in_maps.append({
            "h16": h16,
            "eidx": idx16,
            "edmod": dmod_t,
            "enboth": nb_t,
            "mT": m_shard,
            "wh16": wh16,
            "wm16": wm16,
            "iota": iota,
            "bias": bias,
        })

    nc = _build_program(lay)
    res = bass_utils.run_bass_kernel_spmd(
        nc, in_maps, core_ids=list(range(N_CORES))
    )
    out = np.empty((N_NODES, F), np.float32)
    for i in range(N_CORES):
        out[i * SHARD : (i + 1) * SHARD] = res.results[i]["outT"][:, :SHARD].T
    return out


# revision 25
# speedup vs baseline: 45.8061x; 45.8061x over previous
"""GCN layer (projection + gather/segment-sum + epilogue) on 8 TRN2 cores.

Math: out = relu((segsum(norm[src]*h[src] -> dst) * norm) @ wh + bh + m @ wm + bm)
using (h@wh)*n == (n*h)@wh to hoist both norm scalings into a per-edge weight
normboth[e] = norm[src[e]] * norm[dst[e]] that is folded into the one-hot
scatter matrix. Per core (dst-sharded, 12500 nodes):
  - dma_gather pulls h16[src] rows into edge tiles (128 edges x 128 feats);
    int16 gather indices are local to one of four 32768-row source chunks
  - one-hot scatter matrix built on DVE with one fused tensor_scalar
    (iota == dstmod) * normboth
  - PE: msgs.T @ onehot accumulates agg.T per 128-node dst block in PSUM
  - PE: wh.T @ agg.T + wm.T @ m.T -> out.T, ACT fuses bias+relu
Output is produced feature-major [128, nodes]; host transposes back.
"""

import math

import numpy as np

import concourse.bacc as bacc
import concourse.tile as tile
from concourse import bass, mybir
from concourse import bass_utils

N_NODES = 100000
N_EDGES = 1600000
F = 128
P = 128
N_CORES = 8
SHARD = N_NODES // N_CORES          # 12500 nodes per core
NBLK = math.ceil(SHARD / P)         # 98 blocks of 128 dst nodes
SHARD_PAD = NBLK * P                # 12544
SB_BLOCKS = 7                       # dst blocks per gather super-block
N_SB = math.ceil(NBLK / SB_BLOCKS)  # 14
CHUNK = 32768                       # int16-addressable source rows
N_CHUNKS = math.ceil(N_NODES / CHUNK)
GDT = mybir.dt.float16
GNP = np.float16
BUILD_MODE = "full"  # microbench hook: full | gather | compute | none
BUILD_REPS = 1       # microbench hook: repeat the body R times in one NEFF
N_QUEUES = 4         # SWDGE queues for dma_gather (1..4)
CALL_TILES = 64      # max tiles per dma_gather call
SINGLE_PACKET = False
MSGS_BUFS = 3        # message double/triple buffering


class Layout:
    """Tile-column layout shared by host packing and the device program.
    Cells (dst block, src chunk) are ordered super-block-major, then chunk,
    then block, so each (super-block, chunk) gather call is contiguous."""

    def __init__(self, caps):
        self.caps = caps                      # [NBLK][N_CHUNKS] tile counts
        self.cell_col = np.zeros((NBLK, N_CHUNKS), np.int64)
        self.sb_col = np.zeros(N_SB + 1, np.int64)
        col = 0
        for s in range(N_SB):
            self.sb_col[s] = col
            b_lo, b_hi = s * SB_BLOCKS, min((s + 1) * SB_BLOCKS, NBLK)
            for c in range(N_CHUNKS):
                for b in range(b_lo, b_hi):
                    self.cell_col[b, c] = col
                    col += caps[b, c]
        self.sb_col[N_SB] = col
        self.tot = int(col)
        # rank of each cell id (b * N_CHUNKS + c) in the layout order
        self.rank = np.zeros(NBLK * N_CHUNKS, np.int64)
        self.rank_col = np.zeros(NBLK * N_CHUNKS, np.int64)
        r = 0
        for s in range(N_SB):
            b_lo, b_hi = s * SB_BLOCKS, min((s + 1) * SB_BLOCKS, NBLK)
            for c in range(N_CHUNKS):
                for b in range(b_lo, b_hi):
                    self.rank[b * N_CHUNKS + c] = r
                    self.rank_col[r] = self.cell_col[b, c]
                    r += 1
        self.rank_caps = np.zeros(NBLK * N_CHUNKS, np.int64)
        self.rank_caps[self.rank] = caps.reshape(-1)


def _pack_edges(src, dst, norm_flat):
    core_of = dst // SHARD
    per_core = []
    counts_all = np.zeros((N_CORES, NBLK * N_CHUNKS), np.int64)
    for i in range(N_CORES):
        sel = np.nonzero(core_of == i)[0]
        es = src[sel].astype(np.int64)
        ed = (dst[sel] - i * SHARD).astype(np.int64)
        cell = (ed >> 7) * N_CHUNKS + (es // CHUNK)
        counts_all[i] = np.bincount(cell, minlength=NBLK * N_CHUNKS)
        per_core.append((es, ed, cell))

    caps = ((counts_all.max(axis=0) + P - 1) // P).reshape(NBLK, N_CHUNKS)
    assert caps.sum(axis=1).min() >= 1
    lay = Layout(caps)

    packed = []
    for i in range(N_CORES):
        es, ed, cell = per_core[i]
        rank = lay.rank[cell]
        order = np.argsort(rank, kind="stable")
        es, ed, rank = es[order], ed[order], rank[order]
        counts = np.bincount(rank, minlength=NBLK * N_CHUNKS)
        starts = np.concatenate([[0], np.cumsum(counts)])
        k = np.arange(len(es)) - starts[rank]
        col = lay.rank_col[rank] + (k >> 7)
        row = k & 127
        # int16 gather index local to the source chunk; pads point at row 0
        idx_slots = np.zeros(lay.tot * P, np.int16)
        idx_slots[col * P + row] = (es - (es // CHUNK) * CHUNK).astype(np.int16)
        wrapped = idx_slots.reshape(lay.tot * P // 16, 16).T
        idx16 = np.tile(wrapped, (8, 1))
        dmod = np.full((lay.tot, P), -1.0, np.float32)
        dmod[col, row] = (ed & 127).astype(np.float32)
        nb = np.zeros((lay.tot, P), np.float32)
        nb[col, row] = norm_flat[es] * norm_flat[ed + i * SHARD]
        packed.append((
            np.ascontiguousarray(idx16),
            np.ascontiguousarray(dmod.T),
            np.ascontiguousarray(nb.T),
        ))
    return packed, lay


def _build_program(lay):
    """One SPMD program; all 8 cores run it on their own data."""
    nc = bacc.Bacc(
        "TRN2", target_bir_lowering=False, debug=False, num_devices=N_CORES,
        num_swdge_queues=N_QUEUES,
    )
    f32 = mybir.dt.float32
    tot = lay.tot
    h_d = nc.dram_tensor("h16", [N_NODES, F], GDT, kind="ExternalInput").ap()
    idx_d = nc.dram_tensor("eidx", [P, tot * 8], mybir.dt.int16, kind="ExternalInput").ap()
    dmod_d = nc.dram_tensor("edmod", [P, tot], f32, kind="ExternalInput").ap()
    nb_d = nc.dram_tensor("enboth", [P, tot], f32, kind="ExternalInput").ap()
    mt_d = nc.dram_tensor("mT", [F, SHARD_PAD], f32, kind="ExternalInput").ap()
    wh_d = nc.dram_tensor("wh16", [F, F], GDT, kind="ExternalInput").ap()
    wm_d = nc.dram_tensor("wm16", [F, F], GDT, kind="ExternalInput").ap()
    iota_d = nc.dram_tensor("iota", [P, P], GDT, kind="ExternalInput").ap()
    bias_d = nc.dram_tensor("bias", [F, 1], f32, kind="ExternalInput").ap()
    out_d = nc.dram_tensor("outT", [F, SHARD_PAD], f32, kind="ExternalOutput").ap()

    t_max = int(max(lay.sb_col[s + 1] - lay.sb_col[s] for s in range(N_SB)))

    with tile.TileContext(nc) as tc:
        with (
            tc.tile_pool(name="const", bufs=1) as cpool,
            tc.tile_pool(name="msgs", bufs=1) as mpool,
            tc.tile_pool(name="oh", bufs=16) as ohpool,
            tc.tile_pool(name="agg", bufs=4) as aggpool,
            tc.tile_pool(name="mw", bufs=6) as mwpool,
            tc.tile_pool(name="outp", bufs=4) as opool,
            tc.tile_pool(name="pacc", bufs=4, space="PSUM") as paccp,
            tc.tile_pool(name="pout", bufs=3, space="PSUM") as poutp,
        ):
            idx_s = cpool.tile([P, tot * 8], mybir.dt.int16, tag="idx")
            dmod_s = cpool.tile([P, tot], f32, tag="dmod")
            nb_s = cpool.tile([P, tot], f32, tag="nb")
            wh_s = cpool.tile([F, F], GDT, tag="wh")
            wm_s = cpool.tile([F, F], GDT, tag="wm")
            iota_s = cpool.tile([P, P], GDT, tag="iota")
            bias_s = cpool.tile([F, 1], f32, tag="bias")
            nc.sync.dma_start(out=idx_s[:], in_=idx_d[:])
            nc.sync.dma_start(out=dmod_s[:], in_=dmod_d[:])
            nc.sync.dma_start(out=nb_s[:], in_=nb_d[:])
            nc.sync.dma_start(out=wh_s[:], in_=wh_d[:])
            nc.sync.dma_start(out=wm_s[:], in_=wm_d[:])
            nc.sync.dma_start(out=iota_s[:], in_=iota_d[:])
            nc.sync.dma_start(out=bias_s[:], in_=bias_d[:])

            msgs = [
                mpool.tile([P, t_max * F], GDT, tag=f"m{i}", name=f"msgs{i}")
                for i in range(MSGS_BUFS)
            ]

            do_gather = BUILD_MODE in ("full", "gather")
            do_compute = BUILD_MODE in ("full", "compute")
            if not do_gather:
                nc.vector.memset(msgs[0][:], 0.0)
                nc.vector.memset(msgs[1][:], 0.0)
            gq = [0]
            for s in [s for _ in range(BUILD_REPS) for s in range(N_SB)]:
                b_lo = s * SB_BLOCKS
                b_hi = min((s + 1) * SB_BLOCKS, NBLK)
                sc0 = int(lay.sb_col[s])
                mt = msgs[s % MSGS_BUFS]
                for c in range(N_CHUNKS) if do_gather else []:
                    cc0 = int(lay.cell_col[b_lo, c])
                    ncols_all = int(sum(lay.caps[b, c] for b in range(b_lo, b_hi)))
                    rows = min(CHUNK, N_NODES - c * CHUNK)
                    # SWDGE ring limits one gather to <16384 descriptors
                    for p0 in range(0, ncols_all, CALL_TILES):
                        ncols = min(CALL_TILES, ncols_all - p0)
                        g0 = cc0 + p0
                        ni = ncols * P
                        lo = (g0 - sc0) * F
                        nc.gpsimd.dma_gather(
                            out_ap=mt[:, lo : lo + ncols * F].rearrange(
                                "p (t d) -> p t d", d=F
                            ),
                            in_ap=h_d[c * CHUNK : c * CHUNK + rows, :],
                            idxs_ap=idx_s[:, g0 * 8 : g0 * 8 + ni // 16],
                            num_idxs=ni,
                            num_idxs_reg=ni,
                            elem_size=F,
                            single_packet=SINGLE_PACKET,
                            queue_num=gq[0] % N_QUEUES,
                        )
                        gq[0] += 1

                for b in range(b_lo, b_hi) if do_compute else []:
                    cols = []
                    for c in range(N_CHUNKS):
                        c0 = int(lay.cell_col[b, c])
                        cols.extend(range(c0, c0 + int(lay.caps[b, c])))
                    acc = paccp.tile([F, P], mybir.dt.float32, tag="acc")
                    for j, c_abs in enumerate(cols):
                        oh = ohpool.tile([P, P], GDT, tag="oh")
                        nc.vector.tensor_scalar(
                            out=oh[:],
                            in0=iota_s[:],
                            scalar1=dmod_s[:, c_abs : c_abs + 1],
                            scalar2=nb_s[:, c_abs : c_abs + 1],
                            op0=mybir.AluOpType.is_equal,
                            op1=mybir.AluOpType.mult,
                        )
                        lo = (c_abs - sc0) * F
                        nc.tensor.matmul(
                            acc[:],
                            lhsT=mt[:, lo : lo + F],
                            rhs=oh[:],
                            start=(j == 0),
                            stop=(j == len(cols) - 1),
                        )
                    agg16 = aggpool.tile([F, P], GDT, tag="agg")
                    nc.vector.tensor_copy(out=agg16[:], in_=acc[:])

                    mtf = mwpool.tile([F, P], f32, tag="mtf")
                    nc.sync.dma_start(
                        out=mtf[:], in_=mt_d[:, b * P : (b + 1) * P]
                    )
                    mt16 = mwpool.tile([F, P], GDT, tag="mt16")
                    nc.vector.tensor_copy(out=mt16[:], in_=mtf[:])

                    po = poutp.tile([F, P], mybir.dt.float32, tag="po")
                    nc.tensor.matmul(
                        po[:], lhsT=wh_s[:], rhs=agg16[:], start=True, stop=False
                    )
                    nc.tensor.matmul(
                        po[:], lhsT=wm_s[:], rhs=mt16[:], start=False, stop=True
                    )
                    ot = opool.tile([F, P], f32, tag="ot")
                    nc.scalar.activation(
                        out=ot[:],
                        in_=po[:],
                        func=mybir.ActivationFunctionType.Relu,
                        bias=bias_s[:],
                    )
                    nc.sync.dma_start(
                        out=out_d[:, b * P : (b + 1) * P], in_=ot[:]
                    )
    nc.compile()
    return nc


def kernel(h, m, norm, src, dst, wh, wm, bh, bm):
    h16 = np.ascontiguousarray(np.asarray(h, np.float32).astype(GNP))
    m = np.asarray(m, np.float32)
    norm_flat = np.asarray(norm, np.float32).reshape(-1)
    src = np.asarray(src, np.int32)
    dst = np.asarray(dst, np.int32)
    bias = (np.asarray(bh, np.float32) + np.asarray(bm, np.float32)).reshape(F, 1)

    packed, lay = _pack_edges(src, dst, norm_flat)

    wh16 = np.asarray(wh, np.float32).astype(GNP)
    wm16 = np.asarray(wm, np.float32).astype(GNP)
    iota = np.broadcast_to(np.arange(P, dtype=GNP), (P, P)).copy()

    in_maps = []
    for i in range(N_CORES):
        idx16, dmod_t, nb_t = packed[i]
        m_shard = np.zeros((F, SHARD_PAD), np.float32)
        m_shard[:, :SHARD] = m[i * SHARD : (i + 1) * SHARD].T
        in_maps.append({
            "h16": h16,
            "eidx": idx16,
            "edmod": dmod_t,
            "enboth": nb_t,
            "mT": m_shard,
            "wh16": wh16,
            "wm16": wm16,
            "iota": iota,
            "bias": bias,
        })

    nc = _build_program(lay)
    res = bass_utils.run_bass_kernel_spmd(
        nc, in_maps, core_ids=list(range(N_CORES))
    )
    out = np.empty((N_NODES, F), np.float32)
    for i in range(N_CORES):
        out[i * SHARD : (i + 1) * SHARD] = res.results[i]["outT"][:, :SHARD].T
    return out
